# revision 1
# baseline (speedup 1.0000x reference)
"""DistanceTransformLoss on 8 Trainium2 NeuronCores (Bass/Tile).

loss = BCEWithLogits(predictions, targets).mean()
       + sqrt( sum(pen) / max(count(pen != 0), 1) ),
  pen = (sigmoid(pred) > 0.5) * grassfire_dist_H(targets)

Sharding: data-parallel over batch N (32 images -> 4 per core). Each core
reduces its shard to per-partition partial sums (softplus, p*t, penalty,
count); the host combines the 8 small [128, 128] accumulator tiles in f64.

Per image on a core: load 8 full-width h-stripes (4KB-contiguous DMA rows),
accumulate sum(p*t) per stripe (scalar_tensor_tensor + accum), then per
w-block:
  - PE-transpose p, t 128x128 chunks into PSUM [128w, 1024h]
  - ACT: e = exp(p_T) (fp16); softplus sum via ln(e + 1) with accum_out
    (no Softplus table on this toolchain; Exp+Ln share one act table,
    pre-loaded once so the table-load pass doesn't thrash)
  - DVE grassfire rescaled to u-space so the fwd scan reads raw t_T from
    PSUM: u[i] = max(u[i-1] - 1/1024, t[i]); v = reversed-AP max-scan of
    u; d = 1024*(1 - v). All values are multiples of 2^-10 in [0,1] =>
    exact in fp16. (tensor_tensor_scan is the only scan engine: DVE,
    ~2 cycles/element.)
  - mask m = [e > 1] == [p > 0] (DVE TS); w = 1 - v (DVE TS, 4x);
    pen = m * w on GPSIMD; count indicator via ACT Sign(pen);
    per-h partial sums of pen/ind accumulate across all 32 iterations
    into persistent PSUM [1, 1024] banks via PE matmuls with a ones
    column (start/stop only on first/last iteration).
Host combines partial sums in f64: bce = (sum_sp - sum_pt)/N;
border = 1024*sum_pen_w / max(count, 1); loss = bce + sqrt(border).
"""
import sys

if "/opt/trn_rl_repo" not in sys.path:
    sys.path.insert(0, "/opt/trn_rl_repo")

import numpy as np
from contextlib import ExitStack

import concourse.bass as bass
import concourse.bacc as bacc
import concourse.tile as tile
from concourse import mybir, masks
from concourse.ap import AP
from concourse.bass_utils import run_bass_kernel_spmd
from concourse.hw_specs import get_activation_tables

N_CORES = 8
N_PER_CORE = 4          # 32 images / 8 cores
H = 1024
W = 1024
WB = W // 128           # 8 w-blocks per image
HB = H // 128           # 8 h-blocks
N_ITERS = N_PER_CORE * WB   # 32 iterations per core

F32 = mybir.dt.float32
F16 = mybir.dt.float16
BF16 = mybir.dt.bfloat16

_CACHED_NC = None


def _rev_free(ap):
    """Reverse a 2-D [P, F] AP along the free dim."""
    (pstep, pcount), (fstep, fcount) = ap.ap[0], ap.ap[1]
    return AP(ap.tensor, ap.offset + (fcount - 1) * fstep,
              [[pstep, pcount], [-fstep, fcount]])


def _build_nc():
    nc = bacc.Bacc("TRN2", target_bir_lowering=False, debug=False,
                   enable_asserts=False)
    t_ext = nc.dram_tensor("targets", [N_PER_CORE, H, W], F32,
                           kind="ExternalInput").ap()
    p_ext = nc.dram_tensor("predictions", [N_PER_CORE, H, W], F32,
                           kind="ExternalInput").ap()
    acc_ext = nc.dram_tensor("acc", [128, 4 * N_ITERS], F32,
                             kind="ExternalOutput").ap()
    acc2_ext = nc.dram_tensor("acc2", [1, 2 * H], F32,
                              kind="ExternalOutput").ap()

    with tile.TileContext(nc) as tc, ExitStack() as ctx:
        const_pool = ctx.enter_context(tc.tile_pool(name="const", bufs=1))
        nat_pool = ctx.enter_context(tc.tile_pool(name="nat", bufs=2))
        tr_pool = ctx.enter_context(tc.tile_pool(name="tr", bufs=2))
        sc_pool = ctx.enter_context(tc.tile_pool(name="sc", bufs=4))
        psum_pool = ctx.enter_context(tc.tile_pool(name="ps", bufs=1, space="PSUM"))
        acc_pool = ctx.enter_context(tc.tile_pool(name="acc", bufs=1))

        # Pre-load the one act table containing BOTH Exp and Ln so the
        # table-load pass doesn't alternate tables per activation.
        tables = list(get_activation_tables(nc.m.arch).items())
        set_id = next(i for i, (_, fns) in enumerate(tables)
                      if mybir.ActivationFunctionType.Exp in fns
                      and mybir.ActivationFunctionType.Ln in fns)
        nc.scalar.add_instruction(mybir.InstLoadActFuncSet(
            name=nc.get_next_instruction_name(),
            act_func_set_id=set_id, ins=[], outs=[]))

        idn = const_pool.tile([128, 128], F32, tag="idn")
        masks.make_identity(nc, idn[:])
        dec = const_pool.tile([128, H], F16, tag="dec")
        nc.gpsimd.memset(dec[:], -1.0 / 1024.0)
        ones_col = const_pool.tile([128, 1], F16, tag="ones_col")
        nc.gpsimd.memset(ones_col[:], 1.0)

        accs = acc_pool.tile([128, 4 * N_ITERS], F32)
        nc.vector.memset(accs[:], 0.0)

        # persistent PSUM accumulators: [1, 1024] each (pen sums, counts)
        pacc_pool = ctx.enter_context(
            tc.tile_pool(name="pacc", bufs=1, space="PSUM"))
        pen_acc = pacc_pool.tile([1, H], F32, tag="pen_acc")
        cnt_acc = pacc_pool.tile([1, H], F32, tag="cnt_acc")

        for n in range(N_PER_CORE):
            # full-width h-stripes: 4KB-contiguous DMA rows
            t_img = nat_pool.tile([128, HB * W], F32, tag="t_img")
            p_img = nat_pool.tile([128, HB * W], F32, tag="p_img")
            for hb in range(HB):
                nc.sync.dma_start(
                    t_img[:, hb * W:(hb + 1) * W],
                    t_ext[n, hb * 128:(hb + 1) * 128, :])
                nc.sync.dma_start(
                    p_img[:, hb * W:(hb + 1) * W],
                    p_ext[n, hb * 128:(hb + 1) * 128, :])

            # sum(p * t) per stripe
            for hb in range(HB):
                it = n * HB + hb
                junk2 = tr_pool.tile([128, W], BF16, tag="junk2")
                nc.vector.scalar_tensor_tensor(
                    junk2[:], p_img[:, hb * W:(hb + 1) * W], 0.0,
                    t_img[:, hb * W:(hb + 1) * W],
                    mybir.AluOpType.add, mybir.AluOpType.mult,
                    accum_out=accs[:, N_ITERS + it:N_ITERS + it + 1])

            for wb in range(WB):
                it = n * WB + wb
                c_sp = accs[:, it:it + 1]

                psum_t = psum_pool.tile([128, H], F32, tag="psum_t")
                psum_p = psum_pool.tile([128, H], F32, tag="psum_p")
                for hb in range(HB):
                    off = hb * W + wb * 128
                    nc.tensor.transpose(
                        psum_t[:, hb * 128:(hb + 1) * 128],
                        t_img[:, off:off + 128], idn[:])
                    nc.tensor.transpose(
                        psum_p[:, hb * 128:(hb + 1) * 128],
                        p_img[:, off:off + 128], idn[:])

                e_T = tr_pool.tile([128, H], F16, tag="e")
                sp_junk = tr_pool.tile([128, H], BF16, tag="spj")
                nc.scalar.activation(e_T[:], psum_p[:],
                                     mybir.ActivationFunctionType.Exp)
                nc.scalar.activation(sp_junk[:], e_T[:],
                                     mybir.ActivationFunctionType.Ln,
                                     bias=1.0, accum_out=c_sp)

                # grassfire in u-space: u[i] = max(u[i-1] - 1/1024, t[i]);
                # v = reverse max-scan of u; d = 1024*(1 - v).
                # fwd scan reads t_T straight out of PSUM.
                usc = sc_pool.tile([128, H], F16, tag="usc")
                vsc = sc_pool.tile([128, H], F16, tag="vsc")
                nc.vector.tensor_tensor_scan(
                    usc[:], dec[:], psum_t[:], 0.0,
                    mybir.AluOpType.add, mybir.AluOpType.max)
                nc.vector.tensor_tensor_scan(
                    _rev_free(vsc[:]), dec[:], _rev_free(usc[:]), 0.0,
                    mybir.AluOpType.add, mybir.AluOpType.max)

                m_T = sc_pool.tile([128, H], F16, tag="m")
                w_T = sc_pool.tile([128, H], F16, tag="w")
                pen = sc_pool.tile([128, H], F16, tag="pen")
                ind = sc_pool.tile([128, H], F16, tag="ind")
                nc.vector.tensor_scalar(m_T[:], e_T[:], 1.0, None,
                                        mybir.AluOpType.is_gt)
                # w = 1 - v  (= d / 1024)
                nc.vector.tensor_scalar(w_T[:], vsc[:], -1.0, 1.0,
                                        mybir.AluOpType.mult,
                                        mybir.AluOpType.add)
                nc.gpsimd.tensor_tensor(pen[:], m_T[:], w_T[:],
                                        mybir.AluOpType.mult)
                # ind = [pen > 0] via ACT sign (pen >= 0)
                nc.scalar.activation(ind[:], pen[:],
                                     mybir.ActivationFunctionType.Sign)
                # accumulate per-h sums into PSUM via PE:
                # pen_acc[0, h] += sum_w pen[w, h]  (host multiplies by 1024)
                first, last = (it == 0), (it == N_ITERS - 1)
                for ch in range(2):
                    sl = slice(ch * 512, (ch + 1) * 512)
                    nc.tensor.matmul(pen_acc[:, sl], ones_col[:], pen[:, sl],
                                     start=first, stop=last)
                    nc.tensor.matmul(cnt_acc[:, sl], ones_col[:], ind[:, sl],
                                     start=first, stop=last)

        accs2 = acc_pool.tile([1, 2 * H], F32, tag="accs2")
        nc.scalar.activation(accs2[0:1, 0:H], pen_acc[:],
                             mybir.ActivationFunctionType.Copy)
        nc.scalar.activation(accs2[0:1, H:2 * H], cnt_acc[:],
                             mybir.ActivationFunctionType.Copy)
        nc.sync.dma_start(acc_ext, accs[:])
        nc.sync.dma_start(acc2_ext, accs2[:])

    nc.compile()
    return nc


def _get_nc():
    global _CACHED_NC
    if _CACHED_NC is None:
        _CACHED_NC = _build_nc()
    return _CACHED_NC


def _run(predictions, targets, trace=False, **trace_kwargs):
    """Run the SPMD kernel; returns (loss_scalar, BassKernelResults)."""
    p = np.ascontiguousarray(
        np.asarray(predictions, dtype=np.float32).reshape(32, H, W))
    t = np.ascontiguousarray(
        np.asarray(targets, dtype=np.float32).reshape(32, H, W))

    in_maps = []
    for c in range(N_CORES):
        sl = slice(c * N_PER_CORE, (c + 1) * N_PER_CORE)
        in_maps.append({
            "predictions": np.ascontiguousarray(p[sl]),
            "targets": np.ascontiguousarray(t[sl]),
        })

    nc = _get_nc()
    res = run_bass_kernel_spmd(nc, in_maps, list(range(N_CORES)),
                               trace=trace, **trace_kwargs)

    sum_sp = sum_pt = sum_pen = sum_cnt = 0.0
    for c in range(N_CORES):
        acc = np.asarray(res.results[c]["acc"], dtype=np.float64)
        acc2 = np.asarray(res.results[c]["acc2"], dtype=np.float64)
        sum_sp += acc[:, 0:N_ITERS].sum()
        sum_pt += acc[:, N_ITERS:2 * N_ITERS].sum()
        sum_pen += 1024.0 * acc2[0, 0:H].sum()
        sum_cnt += acc2[0, H:2 * H].sum()

    n_elem = 32.0 * H * W
    bce = (sum_sp - sum_pt) / n_elem
    border = 0.0 if sum_pen == 0.0 else sum_pen / max(sum_cnt, 1.0)
    loss = bce + np.sqrt(border)
    return np.float32(loss), res


def kernel(predictions, targets):
    loss, _ = _run(predictions, targets)
    return np.asarray(loss, dtype=np.float32)

